# revision 11
# baseline (speedup 1.0000x reference)
"""GAT layer kernel for 8 trn2 NeuronCores (v3).

Edges partitioned by src range (12500 nodes/core); within a core, edges
sorted by (64-node src-window w, src). Host pre-gathers node[dst[e]] into
edge order (node_gT, [128=DIN, E_PAD] f16) so the device needs NO
dma_gather: per 128-edge block, h[dst]|sdst come from one matmul with the
node block as stationary operand and W_ext [128,65] moving.

ssrc per edge: per-window telescoping. Own-node scores s come from phase A2;
ds[m] = s[m]-s[m-1] (reset per 64-window) via a block-bidiagonal const
matmul. ssrc[e] = sum_m ds[m] * (e >= start[m]); the is_ge mask is built
with tensor_scalar (per-partition scalar -> DVE 2x packing) and the mask
matmul ACCUMULATES (start=False) into psum col 64, which then holds arg.

exp(LRelu(arg)) via ACT Lrelu then ACT Exp (written straight into the
payload's col 64); payload h*exp is ONE DVE broadcast-mult per 7-block
super-block. Scatter-add per src via one-hot (is_equal vs iota64) matmul
chain into the [64h:64h+64] half of a per-pair PSUM accumulator; finalize
divides by denom+eps once per window pair and DMAs out [128, 64] rows.
"""
import sys
sys.path.insert(0, '/opt/trn_rl_repo')
import numpy as np
import ml_dtypes
from concourse import bacc, library_config
import concourse.bass as bass
import concourse.mybir as mybir
import concourse.tile as tile

F16 = mybir.dt.float16
F32 = mybir.dt.float32
I16 = mybir.dt.int16

EPS = 1e-10
ALPHA = 0.2
W = 64          # src nodes per window
SB = 7          # blocks per psum super-block (7*65 = 455 f32 <= 512)


def build_host_data(node, edge_index, Wm, a, n_cores=8):
    """node [N,128] f32, edge_index [2,E] i32, Wm [128,64] f32, a [128] f32."""
    N, DIN = node.shape
    DOUT = Wm.shape[1]
    NPC = N // n_cores
    Wn = (NPC + W - 1) // W                 # 64-node windows per core
    NODES_PAD = Wn * W
    NPAIR = Wn // 2

    a_src, a_dst = a[:DOUT], a[DOUT:]
    w_dst = Wm @ a_dst
    w_src = Wm @ a_src
    W_ext = np.concatenate(
        [Wm, w_dst[:, None], ALPHA * w_dst[:, None], w_src[:, None]],
        axis=1).astype(np.float16)  # [128, 67]

    # block-bidiagonal difference matrix (64-blocks): out[i] = s[i]-s[i-1],
    # reset at i % 64 == 0.
    Dmat = np.zeros((128, 128), dtype=np.float16)
    Dmat[np.arange(128), np.arange(128)] = 1.0
    for i in range(1, 128):
        if i % W != 0:
            Dmat[i - 1, i] = -1.0

    src = edge_index[0].astype(np.int64)
    dst = edge_index[1].astype(np.int64)

    per_core = []
    cnts = np.zeros((n_cores, Wn), dtype=np.int64)
    for k in range(n_cores):
        m = (src >= k * NPC) & (src < (k + 1) * NPC)
        s = src[m] - k * NPC
        d = dst[m]
        w = s // W
        order = np.lexsort((d, s, w))
        s, d, w = s[order], d[order], w[order]
        per_core.append((s, d, w))
        np.add.at(cnts[k], w, 1)
    NB = np.maximum(1, (cnts.max(axis=0) + 127) // 128)  # [Wn] blocks, baked
    NBMAX = int(NB.max())
    off = np.zeros(Wn, dtype=np.int64)
    off[1:] = np.cumsum(NB[:-1] * 128)
    E_PAD = int((NB * 128).sum())

    meta = dict(N=N, NPC=NPC, Wn=Wn, NODES_PAD=NODES_PAD, E_PAD=E_PAD,
                DOUT=DOUT, NB=NB, off=off, NBMAX=NBMAX, NPAIR=NPAIR)

    node16 = node.astype(np.float16)
    in_maps = []
    for k in range(n_cores):
        s, d, w = per_core[k]
        srel = np.full(E_PAD, -1.0, dtype=np.float16)
        stt = np.full((128, Wn), 30000, dtype=np.float32)
        ngT = np.zeros((128, E_PAD), dtype=np.float16)
        pos = 0
        for wi in range(Wn):
            cnt = int(cnts[k, wi])
            o = int(off[wi])
            h64 = (wi % 2) * W
            sw = s[pos:pos + cnt] - W * wi
            srel[o:o + cnt] = sw.astype(np.float16)
            stt[h64:h64 + W, wi] = np.searchsorted(sw, np.arange(W), side='left')
            ngT[:, o:o + cnt] = node16[d[pos:pos + cnt]].T
            pos += cnt
        srelb = srel.reshape(E_PAD // 128, 128).T.copy()  # [128, E_PAD//128]
        own = np.zeros((DIN, NODES_PAD), dtype=np.float16)
        hi = min((k + 1) * NPC, N)
        own[:, :hi - k * NPC] = node16[k * NPC:hi].T
        in_maps.append({
            "node_gT": ngT, "node_ownT": own, "W_ext": W_ext, "Dmat": Dmat,
            "srelb": srelb, "stt": stt,
        })
    return meta, in_maps


def build_program(meta, n_cores=8):
    Wn, NODES_PAD, E_PAD = meta["Wn"], meta["NODES_PAD"], meta["E_PAD"]
    NB, off, NBMAX, DOUT = meta["NB"], meta["off"], meta["NBMAX"], meta["DOUT"]
    NPAIR = meta["NPAIR"]
    NEE_MAX = NBMAX * 128

    nc = bacc.Bacc("TRN2", target_bir_lowering=False, debug=False,
                   num_devices=n_cores, num_swdge_queues=4)
    ngT_d = nc.dram_tensor("node_gT", [128, E_PAD], F16, kind="ExternalInput")
    own_d = nc.dram_tensor("node_ownT", [128, NODES_PAD], F16, kind="ExternalInput")
    wext_d = nc.dram_tensor("W_ext", [128, 67], F16, kind="ExternalInput")
    dmat_d = nc.dram_tensor("Dmat", [128, 128], F16, kind="ExternalInput")
    srelb_d = nc.dram_tensor("srelb", [128, E_PAD // 128], F16, kind="ExternalInput")
    stt_d = nc.dram_tensor("stt", [128, Wn], F32, kind="ExternalInput")
    out_d = nc.dram_tensor("out", [NODES_PAD, DOUT], F32, kind="ExternalOutput")

    NWB = NODES_PAD // 128  # 128-node blocks (= window pairs)

    with tile.TileContext(nc) as tc:
        with (tc.tile_pool(name="const", bufs=1) as cpool,
              tc.tile_pool(name="xin", bufs=3) as xpool,
              tc.tile_pool(name="masks", bufs=3) as mpool,
              tc.tile_pool(name="mid", bufs=8) as midp,
              tc.tile_pool(name="psA2", bufs=1, space="PSUM") as psA2,
              tc.tile_pool(name="psH", bufs=4, space="PSUM") as psH,
              tc.tile_pool(name="psAcc", bufs=2, space="PSUM") as psAcc):

            wext_t = cpool.tile([128, 67], F16)
            nc.sync.dma_start(out=wext_t[:], in_=wext_d[:])
            dmat_t = cpool.tile([128, 128], F16)
            nc.sync.dma_start(out=dmat_t[:], in_=dmat_d[:])
            stt_t = cpool.tile([128, Wn], F32)
            nc.sync.dma_start(out=stt_t[:], in_=stt_d[:])
            srelb_t = cpool.tile([128, E_PAD // 128], F16)
            nc.sync.dma_start(out=srelb_t[:], in_=srelb_d[:])
            own_t = cpool.tile([128, NODES_PAD], F16)
            nc.sync.dma_start(out=own_t[:], in_=own_d[:])
            iota64 = cpool.tile([128, W], F16)
            nc.gpsimd.iota(iota64[:], pattern=[[1, W]], base=0,
                           channel_multiplier=0,
                           allow_small_or_imprecise_dtypes=True)
            iota_run = cpool.tile([128, NEE_MAX], I16)
            nc.gpsimd.iota(iota_run[:], pattern=[[1, NEE_MAX]], base=0,
                           channel_multiplier=0,
                           allow_small_or_imprecise_dtypes=True)

            # ---------------- phase A2: own-node scores -> ds ----------------
            ps_s = psA2.tile([128, NWB], F32, tag="ps_s")
            for c in range(NWB):
                nc.tensor.matmul(ps_s[:, c:c + 1],
                                 lhsT=own_t[:, c * 128:(c + 1) * 128],
                                 rhs=wext_t[:, 66:67], start=True, stop=True)
            s2sb = cpool.tile([128, NWB], F16)
            nc.vector.tensor_copy(out=s2sb[:], in_=ps_s[:])
            ps_ds = psA2.tile([128, NWB], F32, tag="ps_ds")
            nc.tensor.matmul(ps_ds[:], lhsT=dmat_t[:], rhs=s2sb[:],
                             start=True, stop=True)
            dsb2 = cpool.tile([128, NWB, 2], F16)
            nc.vector.tensor_copy(out=dsb2[:, :, 0], in_=ps_ds[:])
            nc.vector.tensor_scalar_mul(dsb2[:, :, 1], ps_ds[:], float(ALPHA))

            # ---------------- main loop (per window pair) ----------------
            # scatter matmuls are emitted one super-block late so the
            # (in-order) PE never waits on the DVE payload of the same sb.
            from bass_rust import AP as _AP
            pending = []

            def flush_one():
                fn, fin = pending.pop(0)
                fn()
                if fin is not None:
                    fin()

            for p in range(NPAIR):
                acc_ps = psAcc.tile([128, 65], F32, tag="acc")

                def mk_fin(acc_ps=acc_ps, p=p):
                    def fin():
                        den = midp.tile([128, 1], F32, tag="den")
                        nc.vector.tensor_scalar_add(den[:], acc_ps[:, 64:65],
                                                    float(EPS))
                        rec = midp.tile([128, 1], F32, tag="rec")
                        nc.vector.reciprocal(rec[:], den[:])
                        ob = midp.tile([128, 64], F32, tag="ob")
                        nc.scalar.mul(ob[:], acc_ps[:, 0:64], rec[:])
                        nc.sync.dma_start(out=out_d[p * 128:(p + 1) * 128, :],
                                          in_=ob[:])
                    return fin

                for h in range(2):
                    w = 2 * p + h
                    nb = int(NB[w])
                    nee = nb * 128
                    o = int(off[w])
                    col = o // 128
                    h64 = h * W

                    xt = xpool.tile([128, NEE_MAX], F16, tag="xt")
                    nc.sync.dma_start(out=xt[:, :nee], in_=ngT_d[:, o:o + nee])

                    u2 = mpool.tile([128, NBMAX, W], F16, tag="u2")
                    i2 = iota64[:].unsqueeze(1)
                    i2b = _AP(tensor=i2.tensor, offset=i2.offset,
                              ap=[i2.ap[0], [0, nb], [1, W]])
                    nc.vector.tensor_tensor(
                        out=u2[:, :nb, :],
                        in0=srelb_t[:, col:col + nb].unsqueeze(2).to_broadcast([128, nb, W]),
                        in1=i2b, op=mybir.AluOpType.is_equal)

                    ut = mpool.tile([128, NEE_MAX], F16, tag="ut")
                    nc.vector.tensor_scalar(
                        out=ut[:, :nee], in0=iota_run[:, :nee],
                        scalar1=stt_t[:, w:w + 1], scalar2=None,
                        op0=mybir.AluOpType.is_ge)

                    for s0 in range(0, nb, SB):
                        ns = min(SB, nb - s0)
                        ps = psH.tile([128, SB, 66], F32, tag="ps")
                        for bi in range(ns):
                            b = s0 + bi
                            nc.tensor.matmul(ps[:, bi, :],
                                             lhsT=xt[:, b * 128:(b + 1) * 128],
                                             rhs=wext_t[:, 0:66],
                                             start=(bi == 0), stop=False)
                        for bi in range(ns):
                            b = s0 + bi
                            nc.tensor.matmul(ps[:, bi, 64:66],
                                             lhsT=ut[h64:h64 + W, b * 128:(b + 1) * 128],
                                             rhs=dsb2[h64:h64 + W, p, :],
                                             start=False, stop=(bi == ns - 1))
                        xs = midp.tile([128, SB, 2], F16, tag="xs")
                        nc.scalar.activation(xs[:, :ns, :], ps[:, :ns, 64:66],
                                             mybir.ActivationFunctionType.Exp)
                        pt = midp.tile([128, SB, 65], F16, tag="pt")
                        nc.vector.tensor_tensor(out=pt[:, :ns, 64],
                                                in0=xs[:, :ns, 0],
                                                in1=xs[:, :ns, 1],
                                                op=mybir.AluOpType.max)
                        nc.vector.tensor_tensor(
                            out=pt[:, :ns, 0:64], in0=ps[:, :ns, 0:64],
                            in1=pt[:, :ns, 64:65].to_broadcast([128, ns, 64]),
                            op=mybir.AluOpType.mult)

                        def mk_scatter(u2=u2, pt=pt, acc_ps=acc_ps, s0=s0,
                                       ns=ns, nb=nb, h64=h64):
                            def emit():
                                for bi in range(ns):
                                    b = s0 + bi
                                    nc.tensor.matmul(acc_ps[h64:h64 + W, :],
                                                     lhsT=u2[:, b, :],
                                                     rhs=pt[:, bi, :],
                                                     start=(b == 0),
                                                     stop=(b == nb - 1))
                            return emit

                        last = (h == 1) and (s0 + SB >= nb)
                        if len(pending) >= 1:
                            flush_one()
                        pending.append((mk_scatter(), mk_fin() if last else None))

            while pending:
                flush_one()

    nc.compile()
    return nc


def run(node, edge_index, Wm, a, n_cores=8, trace=False):
    from concourse.bass_utils import run_bass_kernel_spmd
    meta, in_maps = build_host_data(node, edge_index, Wm, a, n_cores)
    nc = build_program(meta, n_cores)
    res = run_bass_kernel_spmd(nc, in_maps, core_ids=list(range(n_cores)), trace=trace)
    NPC = meta["NPC"]
    out = np.concatenate([res.results[k]["out"][:NPC] for k in range(n_cores)], axis=0)
    return out, res, meta


_CACHE = {}


def kernel(node, edge_index, W, a):
    """Full inputs -> full output [100000, 64] f32, computed on 8 NeuronCores."""
    from concourse.bass_utils import run_bass_kernel_spmd
    node = np.asarray(node, dtype=np.float32)
    edge_index = np.asarray(edge_index, dtype=np.int32)
    W = np.asarray(W, dtype=np.float32)
    a = np.asarray(a, dtype=np.float32)
    n_cores = 8
    meta, in_maps = build_host_data(node, edge_index, W, a, n_cores)
    key = (node.shape, edge_index.shape, meta["E_PAD"],
           tuple(meta["NB"].tolist()))
    if key in _CACHE:
        nc = _CACHE[key]
    else:
        nc = build_program(meta, n_cores)
        _CACHE[key] = nc
    res = run_bass_kernel_spmd(nc, in_maps, core_ids=list(range(n_cores)))
    NPC = meta["NPC"]
    out = np.concatenate([res.results[k]["out"][:NPC] for k in range(n_cores)], axis=0)
    return out.astype(np.float32)
